# revision 81
# baseline (speedup 1.0000x reference)
"""Biaffine edge attention on 8 Trainium2 NeuronCores.

Math (per batch b):
    out[i,o] = head[i,:] @ U @ dep[o,:] + head[i,:]@wh + dep[o,:]@wd + b
with head/dep [S=2048, D=256], U [D,D], edge_W = [wh | wd] (each [D]).

Sharding: pure data-parallel over batch B=8 -> one batch per core,
constants replicated. No collectives.

Per-core kernel (bf16 matmul operands, f32 PSUM accumulate, bf16 output
stores; harness gate is rel_err < 2e-2 and this lands ~4e-3):
    ATf[e,i] = sum_d U[d,e] * headT[d,i] + wd[e]      (the dep-side rank-1
               term ds[o] rides the e-contraction for free)
    hs[i]    = sum_d headT[d,i] * whT[d] + b          (32 tiny PE matmuls
               into one PSUM strip; per-partition bias in the epilogue)
    out[i,o] = sum_e ATf[e,i] * depT[e,o]  + hs[i]

Key moves vs a straightforward f32 kernel:
  - bf16 everywhere on-chip: PE matmuls at 1 cyc/row, and the output
    stores in bf16 halve the dominant store traffic (upcast to f32
    host-side, ~4e-3 rel err vs the 2e-2 gate).
  - Column-major sweep: out is produced in [256,512] row-pair stores,
    column-stripe by column-stripe in dep-load-arrival order, so the
    store train starts ~9us in and the PE never waits for the last dep
    load. Head transposes / ATf / hs drop into the col-0 sweep as their
    loads land; everything is PE-paced from there.
  - Loads are gpsimd/SWDGE f32->bf16 cast-loads: HBM read traffic is
    unchanged but on-chip tiles halve and PE transposes run at
    1 cyc/row. A dummy f32 matmul at t~1us absorbs the PE p-state ramp
    (4x slower clock until ~3us of continuous busy).
  - DO NOT DMA into float32r-typed tiles: it hard-faults the exec unit
    (NRT_EXEC_UNIT_UNRECOVERABLE); f32r operands must be written by a
    compute op. (bf16 operands sidestep the issue entirely.)
"""

import numpy as np

import concourse.bass as bass
import concourse.tile as tile
from concourse import bacc, mybir
from concourse.bass_utils import run_bass_kernel_spmd

B, S, D = 8, 2048, 256
P = 128          # partitions
OC = 512         # output free-dim chunk (one PSUM bank of fp32)
GB = 4           # row-blocks per input load group
NG = S // (P * GB)   # 4 load groups per input
NI = S // P      # 16 row blocks
NO = S // OC     # 4 output column chunks
ND = D // P      # 2 contraction chunks
F32 = mybir.dt.float32
BF16 = mybir.dt.bfloat16

# pack layout (free-dim col offsets in the [P, PACKW] f32 constant tensor)
EYE_O = 0                # [P, P] identity
U_O = EYE_O + P          # [P, ND*D]: U rows dc*128.. as two [128,256] blocks
WHT_O = U_O + ND * D     # [P, ND] whT[p, dc] = wh[dc*128+p]
WDT_O = WHT_O + ND       # [P, ND] wdT[p, dc] = wd[dc*128+p]
B_O = WDT_O + ND         # [P, 1] bias replicated
PACKW = B_O + 1 + 3      # pad to 648


def build_nc(reps=1, unroll=False, hwdge_loads=False, quads=False, obufs=8,
             hs_in_pst=False, dual_store=False, pool_cvt=False,
             pst_bufs=3):
    """reps>1 wraps the body in a HW For_i loop -- used only for timing.
    unroll=N duplicates the body python-side (TimelineSim can't branch).
    Default loads are gpsimd/SWDGE f32->bf16 cast-loads (interleaved A/B:
    ~2us/rep faster than sync/HWDGE f32 loads, and bf16 transposes run at
    1 cyc/row); hwdge_loads=True switches to sync-queue f32 loads."""
    nc = bacc.Bacc("TRN2", target_bir_lowering=False, debug=False, num_devices=B)

    head_d = nc.dram_tensor("head", [S, D], F32, kind="ExternalInput")
    dep_d = nc.dram_tensor("dep", [S, D], F32, kind="ExternalInput")
    pack_d = nc.dram_tensor("pack", [P, PACKW], F32, kind="ExternalInput")
    out_d = nc.dram_tensor("out", [S, S], BF16, kind="ExternalOutput")

    Ident = mybir.ActivationFunctionType.Identity

    with tile.TileContext(nc) as tc:
        with (
            tc.tile_pool(name="const", bufs=1) as cpool,
            tc.tile_pool(name="persist", bufs=1) as ppool,
            tc.tile_pool(name="stage", bufs=10) as stage,
            tc.tile_pool(name="outbuf", bufs=obufs) as outbuf,
            tc.tile_pool(name="ps_t", bufs=pst_bufs,
                         space=bass.MemorySpace.PSUM) as ps_t,
            tc.tile_pool(name="ps_mm", bufs=8 - pst_bufs,
                         space=bass.MemorySpace.PSUM) as ps_mm,
        ):
            # ---- PE p-state warmup: one strict-f32 matmul at cold clock
            # runs ~3.4us -- the continuous-busy credit the PE needs ----
            warm = cpool.tile([P, OC], F32, name="warm", tag="warm")
            nc.vector.memset(warm[:], 1.0)
            pw = ps_mm.tile([P, OC], F32, name="psmm", tag="psmm")
            nc.tensor.matmul(pw[:], warm[:, 0:P], warm[:], start=True, stop=True)

            # ---- constants: ONE packed f32 DMA on sync, bf16 copies ----
            pack = cpool.tile([P, PACKW], F32, name="pack", tag="pack")
            nc.sync.dma_start(pack[:], pack_d[:])
            wdT = pack[:, WDT_O:WDT_O + ND]          # f32 bias APs are fine
            b128 = pack[:, B_O:B_O + 1]
            eye = cpool.tile([P, P], BF16, name="eye", tag="eye")
            nc.vector.tensor_copy(eye[:], pack[:, EYE_O:EYE_O + P])
            whT = cpool.tile([P, ND], BF16, name="whT", tag="whT")
            nc.vector.tensor_copy(whT[:], pack[:, WHT_O:WHT_O + ND])
            u_sb = []
            for dc in range(ND):
                u_t = cpool.tile([P, D], BF16, name=f"u{dc}", tag=f"u{dc}")
                nc.vector.tensor_copy(u_t[:], pack[:, U_O + dc * D:U_O + (dc + 1) * D])
                u_sb.append(u_t)

            # ---- persistent SBUF tensors (all bf16) ----
            # one PSUM strip holds the 16 hs accumulators for the whole
            # body; hs_in_pst carves it from the transpose pool (idle
            # during the store train) instead of a ps_mm slot
            if hs_in_pst:
                hs_ps = ps_t.tile([P, OC], F32, name="pst", tag="pst")
            else:
                hs_ps = ps_mm.tile([P, OC], F32, name="psmm", tag="psmm")

            headT = [ppool.tile([P, S], BF16, name=f"headT{dc}", tag=f"headT{dc}")
                     for dc in range(ND)]
            depT = [ppool.tile([P, S], BF16, name=f"depT{dc}", tag=f"depT{dc}")
                    for dc in range(ND)]
            atf = [ppool.tile([P, S], BF16, name=f"atf{eb}", tag=f"atf{eb}")
                   for eb in range(ND)]
            hs_colb = ppool.tile([P, NI], F32, name="hs_colb", tag="hs_colb")

            eng_ctr = [0]

            def load_group(src_dram, g):
                # [128, GB*D] bf16, cast from f32 in the DMA (gpsimd/SWDGE),
                # or raw f32 via sync/HWDGE with hwdge_loads=True
                dt = F32 if hwdge_loads else BF16
                nat = stage.tile([P, GB * D], dt, name="nat", tag="nat")
                src = src_dram[g * GB * P:(g + 1) * GB * P, :]
                src3 = src.rearrange("(j p) d -> p j d", p=P)
                eng = nc.sync if hwdge_loads else nc.gpsimd
                eng.dma_start(nat[:].rearrange("p (j d) -> p j d", d=D), src3)
                if pool_cvt and hwdge_loads:
                    # convert to bf16 on the idle Pool engine so the PE
                    # transposes run at 1 cyc/row instead of f32's 2
                    natb = stage.tile([P, GB * D], BF16, name="natb", tag="natb")
                    nc.gpsimd.tensor_copy(natb[:], nat[:])
                    return natb
                return nat

            def transpose_group(nat, dstT, g):
                # 8 PE transposes (bf16, 1cyc/row) -> [128,512] PSUM ->
                # bf16 collect copies alternating DVE/ACT
                tdt = nat.dtype
                teye = pack[:, EYE_O:EYE_O + P] if tdt == F32 else eye[:]
                for dc in range(ND):
                    pst = ps_t.tile([P, GB * P], tdt, name="pst", tag="pst")
                    for j in range(GB):
                        nc.tensor.transpose(
                            pst[:, j * P:(j + 1) * P],
                            nat[:, j * D + dc * P: j * D + dc * P + P],
                            teye,
                        )
                    dst = dstT[dc][:, g * GB * P:(g + 1) * GB * P]
                    eng_ctr[0] += 1
                    if eng_ctr[0] % 2 == 0:
                        nc.vector.tensor_copy(dst, pst[:])
                    else:
                        nc.scalar.copy(dst, pst[:])

            def hs_group(g):
                # hs for this group's 4 row-blocks on the PE: per block,
                # 2 accumulating [128d x 128i]^T @ [128d x 1] matmuls with
                # whT as the moving vector -> hs_ps column; then one ACT
                # bias. ~100ns each, absorbed in PE's load-phase slack --
                # no DVE/Pool work at all.
                for ib in range(g * GB, (g + 1) * GB):
                    for dc in range(ND):
                        nc.tensor.matmul(
                            hs_ps[:, ib:ib + 1],
                            headT[dc][:, ib * P:(ib + 1) * P],
                            whT[:, dc:dc + 1],
                            start=(dc == 0),
                            stop=(dc == ND - 1),
                        )
                nc.scalar.activation(
                    hs_colb[:, g * GB:(g + 1) * GB],
                    hs_ps[:, g * GB:(g + 1) * GB], Ident, bias=b128
                )

            def atf_group(g):
                # ATf chunk ic=g from headT[:, g*512:(g+1)*512]
                for eb in range(ND):
                    pa = ps_mm.tile([P, OC], F32, name="psmm", tag="psmm")
                    for dc in range(ND):
                        nc.tensor.matmul(
                            pa[:],
                            u_sb[dc][:, eb * P:(eb + 1) * P],
                            headT[dc][:, g * OC:(g + 1) * OC],
                            start=(dc == 0),
                            stop=(dc == ND - 1),
                        )
                    # split the two bias/round chunks across ACT and DVE
                    if eb == 0:
                        nc.scalar.activation(
                            atf[eb][:, g * OC:(g + 1) * OC], pa[:], Ident,
                            bias=wdT[:, eb:eb + 1],
                        )
                    else:
                        nc.vector.tensor_scalar_add(
                            atf[eb][:, g * OC:(g + 1) * OC], pa[:],
                            wdT[:, eb:eb + 1],
                        )

            def out_pair(ib, oc, split_store=False, nrow=2):
                # nrow row-blocks x one col chunk -> one [128,nrow*512] tile
                # -> ONE store to the contiguous [nrow*128,512] DRAM region
                # (fewer per-store dispatches + sem round-trips than one
                # store per chunk). split_store=True (very last group)
                # stores each half as soon as its epilogue lands.
                ot = outbuf.tile([P, nrow * OC], BF16, name="ot", tag="ot")
                for j in range(nrow):
                    po = ps_mm.tile([P, OC], F32, name="psmm", tag="psmm")
                    for eb in range(ND):
                        nc.tensor.matmul(
                            po[:],
                            atf[eb][:, (ib + j) * P:(ib + j + 1) * P],
                            depT[eb][:, oc * OC:(oc + 1) * OC],
                            start=(eb == 0),
                            stop=(eb == ND - 1),
                        )
                    dst = ot[:, j * OC:(j + 1) * OC]
                    if (ib + j + oc) % 2 == 0:
                        nc.scalar.activation(
                            dst, po[:], Ident, bias=hs_colb[:, ib + j:ib + j + 1]
                        )
                    else:
                        nc.vector.tensor_scalar_add(
                            dst, po[:], hs_colb[:, ib + j:ib + j + 1]
                        )
                    if split_store:
                        nc.sync.dma_start(
                            out_d[(ib + j) * P:(ib + j + 1) * P,
                                  oc * OC:(oc + 1) * OC],
                            dst,
                        )
                if not split_store:
                    dram = out_d[ib * P:(ib + nrow) * P, oc * OC:(oc + 1) * OC]
                    seng = nc.scalar if (dual_store and (ib // nrow) % 2) else nc.sync
                    seng.dma_start(
                        dram.rearrange("(j p) d -> p j d", p=P),
                        ot[:].rearrange("p (j d) -> p j d", d=OC),
                    )

            def body():
                # ---- loads in first-use order: col0 needs all heads plus
                # dep0; deps 1-3 are only touched from col1 (~t+15us) on ----
                nat_h, nat_p = {}, {}
                nat_h[0] = load_group(head_d, 0)
                nat_p[0] = load_group(dep_d, 0)
                for g in range(1, NG):
                    nat_h[g] = load_group(head_d, g)
                for g in range(1, NG):
                    nat_p[g] = load_group(dep_d, g)

                # ---- col-major sweep; head blocks + dep transposes drop in
                # as their loads land. Each column's dep transpose is hoisted
                # into the middle of the PREVIOUS column's sweep so depT[oc]
                # is ready before the boundary (no store-train stall). ----
                transpose_group(nat_h[0], headT, 0)
                atf_group(0)
                hs_group(0)
                transpose_group(nat_p[0], depT, 0)
                for ib in range(0, NI, 2):
                    k = ib // GB
                    if ib % GB == 0 and k > 0:
                        transpose_group(nat_h[k], headT, k)
                        atf_group(k)
                        hs_group(k)
                    out_pair(ib, 0)
                    if ib == 8:
                        transpose_group(nat_p[1], depT, 1)
                step = 4 if quads else 2
                for oc in range(1, NO):
                    for ib in range(0, NI, step):
                        out_pair(ib, oc, nrow=step,
                                 split_store=(oc == NO - 1 and ib == NI - step))
                        if ib == 6 and oc < NO - 1:
                            transpose_group(nat_p[oc + 1], depT, oc + 1)

            nbody = int(unroll) if unroll else 1
            if reps > 1:
                with tc.For_i(0, reps, 1):
                    for _ in range(nbody):
                        body()
            else:
                for _ in range(nbody):
                    body()

    nc.finalize()
    return nc


_NC_CACHE = {}


def _get_nc(reps=1):
    if reps not in _NC_CACHE:
        _NC_CACHE[reps] = build_nc(reps)
    return _NC_CACHE[reps]


def make_in_maps(head, dep, edge_U, edge_W, edge_b):
    head = np.ascontiguousarray(np.asarray(head, dtype=np.float32))
    dep = np.ascontiguousarray(np.asarray(dep, dtype=np.float32))
    u = np.asarray(edge_U, dtype=np.float32)
    w = np.asarray(edge_W, dtype=np.float32).reshape(-1)
    wh, wd = w[:D], w[D:]
    pack = np.zeros((P, PACKW), np.float32)
    pack[:, EYE_O:EYE_O + P] = np.eye(P, dtype=np.float32)
    for dc in range(ND):
        pack[:, U_O + dc * D:U_O + (dc + 1) * D] = u[dc * P:(dc + 1) * P, :]
    pack[:, WHT_O:WHT_O + ND] = wh.reshape(ND, P).T
    pack[:, WDT_O:WDT_O + ND] = wd.reshape(ND, P).T
    pack[:, B_O] = float(np.asarray(edge_b).reshape(-1)[0])
    return [
        {"head": head[b], "dep": dep[b], "pack": pack}
        for b in range(B)
    ]


def kernel(head, dep, edge_U, edge_W, edge_b):
    nc = _get_nc()
    in_maps = make_in_maps(head, dep, edge_U, edge_W, edge_b)
    res = run_bass_kernel_spmd(nc, in_maps, core_ids=list(range(B)))
    return np.stack(
        [np.asarray(res.results[b]["out"]).astype(np.float32) for b in range(B)],
        axis=0,
    )
